# revision 2
# baseline (speedup 1.0000x reference)
"""Trainium2 Bass kernel for nn_CumsumInterpolationEmbedding.

    E = LayerNorm(cumsum(cumsum_embedding, axis=0)) * gamma + beta
        + [alpha, 1-alpha] @ interpolation_embedding     # alpha_n = (N-1-n)/(N-1)
    out[b, l, :] = E[index_tensor[b, l], :]              # [4096, 200, 128] f32

Data-parallel over the 4096 batch across 8 NeuronCores; each core runs an
identical program on its own 1/8 of the index stream (102,400 lookups).

Per-core device program:
  prologue — build E [1024, 128] once (PE cumsum via triangular/ones matmuls,
    DVE bn_stats/bn_aggr layernorm, ACT rsqrt, K=3 matmul for interpolation +
    beta), then park E in a DRAM scratch tensor for the gathers.
  main loop — J chunks of C=6400 lookups:
    * gpsimd dma_gather: 6400 descriptors of 512 B each, E rows -> SBUF
      tile [128, G=50, 128] (row i of the chunk lands on partition i%128)
    * HWDGE writeout SBUF -> the per-core contiguous output slice.

The 102,400 x 512 B gather descriptors are the dominant cost, and a single
SWDGE queue is latency-limited at ~8.5 ns/row (measured): the DMA engines
allow only a small number of outstanding transfers per ring. Spreading the
gather chunks round-robin over 4 SWDGE queues (num_swdge_queues=4, the ucode
max) quadruples ring-level concurrency and brings the gather to ~2.1 ns/row,
at which point it overlaps with the writeout near the HBM write roofline.

Each dma_gather chunk is processed by one gpsimd core pair (pair q serves
queue q) which reads the chunk's indices from ITS OWN 32 partitions of the
index tile; the host packs chunk j's index columns into partition block
32*(j%4)..32*(j%4)+31 (two 16-partition replicas), so the index tile is
[128, rows/16/4] int16 -- 4x smaller than replicating across all 8 pairs.

The host pre-permutes each core's index stream so that partition p of chunk j
holds output rows jC + p*G .. jC + p*G + G-1; the writeout is then 128
descriptors of G*512 B contiguous DRAM each (full DMA line rate), and the
gathered rows land in exactly original output order. Host work is only
casting/reordering the int64 indices (no arithmetic on the data path).
"""

import numpy as np

import concourse.bacc as bacc
import concourse.bass as bass
import concourse.mybir as mybir
from concourse._compat import get_trn_type
from concourse.bass_utils import run_bass_kernel_spmd
from concourse.library_config import mlp

F32 = mybir.dt.float32
I16 = mybir.dt.int16
AF = mybir.ActivationFunctionType
ALU = mybir.AluOpType

N_CORES = 8
N = 1024            # embedding table rows
D = 128             # embedding dim
EPS = 1e-5
B, L = 4096, 200
R = B * L // N_CORES  # 102400 output rows per core
G = 50                # output rows per partition per gather chunk
C = 128 * G           # 6400 indices per gather chunk
NB = 6                # gather buffers in flight
NQ = 4                # SWDGE queues (ucode max); chunk j uses queue j % NQ
TC = N // 128         # 8 table chunks


def build_nc(rows: int = R, reps: int = 1) -> bass.Bass:
    """reps > 1 repeats the main gather/writeout loop (idempotent rewrites of
    the same output) so device time can be measured as a slope without NTFF."""
    J = rows // C
    assert J * C == rows and rows % 16 == 0
    assert J % NQ == 0
    JJ = J * reps
    IXC = rows // 16 // NQ  # idx columns per queue block

    nc = bacc.Bacc(get_trn_type() or "TRN2", num_swdge_queues=NQ)
    csemb = nc.dram_tensor("csemb", [N, D], F32, kind="ExternalInput")
    triu = nc.dram_tensor("triu", [128, 128], F32, kind="ExternalInput")
    onesm = nc.dram_tensor("onesm", [128, 128], F32, kind="ExternalInput")
    a3t = nc.dram_tensor("a3t", [3, N], F32, kind="ExternalInput")
    i3 = nc.dram_tensor("i3", [3, D], F32, kind="ExternalInput")
    gammab = nc.dram_tensor("gammab", [128, D], F32, kind="ExternalInput")
    idx16 = nc.dram_tensor("idx16", [128, IXC], I16, kind="ExternalInput")
    out = nc.dram_tensor("out", [rows, D], F32, kind="ExternalOutput")
    etab = nc.dram_tensor("etab", [N, D], F32)

    import contextlib

    with contextlib.ExitStack() as ctx:
        block = ctx.enter_context(nc.Block())
        cs_sb = ctx.enter_context(nc.sbuf_tensor("cs_sb", [128, TC, D], F32))
        triu_sb = ctx.enter_context(nc.sbuf_tensor("triu_sb", [128, 128], F32))
        ones_sb = ctx.enter_context(nc.sbuf_tensor("ones_sb", [128, 128], F32))
        a3t_sb = ctx.enter_context(nc.sbuf_tensor("a3t_sb", [3, N], F32))
        i3_sb = ctx.enter_context(nc.sbuf_tensor("i3_sb", [3, D], F32))
        gamma_sb = ctx.enter_context(nc.sbuf_tensor("gamma_sb", [128, D], F32))
        e_sb = ctx.enter_context(nc.sbuf_tensor("e_sb", [128, TC, D], F32))
        stats_sb = ctx.enter_context(nc.sbuf_tensor("stats_sb", [128, TC, 6], F32))
        mv_sb = ctx.enter_context(nc.sbuf_tensor("mv_sb", [128, TC, 2], F32))
        rstd_sb = ctx.enter_context(nc.sbuf_tensor("rstd_sb", [128, TC], F32))
        eps_sb = ctx.enter_context(nc.sbuf_tensor("eps_sb", [128, 1], F32))
        idx_sb = ctx.enter_context(nc.sbuf_tensor("idx_sb", [128, IXC], I16))
        bufs = [
            ctx.enter_context(nc.sbuf_tensor(f"buf{b}", [128, G * D], F32))
            for b in range(NB)
        ]
        ps_cs = ctx.enter_context(nc.psum_tensor("ps_cs", [128, TC, D], F32))
        ps_in = ctx.enter_context(nc.psum_tensor("ps_in", [128, TC, D], F32))

        s_in = ctx.enter_context(nc.semaphore("s_in"))
        s_ix = ctx.enter_context(nc.semaphore("s_ix"))
        s_pe = ctx.enter_context(nc.semaphore("s_pe"))
        s_pi = ctx.enter_context(nc.semaphore("s_pi"))
        s_mv = ctx.enter_context(nc.semaphore("s_mv"))
        s_rs = ctx.enter_context(nc.semaphore("s_rs"))
        s_e = ctx.enter_context(nc.semaphore("s_e"))
        s_et = ctx.enter_context(nc.semaphore("s_et"))
        s_eps = ctx.enter_context(nc.semaphore("s_eps"))
        s_dv = ctx.enter_context(nc.semaphore("s_dv"))
        g_sems = [ctx.enter_context(nc.semaphore(f"s_g{b}")) for b in range(NB)]
        w_sems = [ctx.enter_context(nc.semaphore(f"s_w{b}")) for b in range(NB)]

        @block.sync
        def _(sp: bass.BassEngine):
            sp.dma_start(cs_sb[:, :, :], csemb[:, :].rearrange("(c p) d -> p c d", p=128)).then_inc(s_in, 16)
            sp.dma_start(triu_sb[:, :], triu[:, :]).then_inc(s_in, 16)
            sp.dma_start(ones_sb[:, :], onesm[:, :]).then_inc(s_in, 16)
            sp.dma_start(a3t_sb[:, :], a3t[:, :]).then_inc(s_in, 16)
            sp.dma_start(i3_sb[:, :], i3[:, :]).then_inc(s_in, 16)
            sp.dma_start(gamma_sb[:, :], gammab[:, :]).then_inc(s_in, 16)
            sp.dma_start(idx_sb[:, :], idx16[:, :]).then_inc(s_ix, 16)

            # park the finished embedding matrix in DRAM for the gathers
            sp.wait_ge(s_e, TC)
            sp.dma_start(
                etab[:, :].rearrange("(c p) d -> p c d", p=128), e_sb[:, :, :]
            ).then_inc(s_et, 16)

            # writeouts: chunk j -> output rows [jC, (j+1)C), 128 descriptors
            # of G*512 contiguous bytes each
            for jj in range(JJ):
                j, b, r = jj % J, jj % NB, jj // NB
                sp.wait_ge(g_sems[b], 16 * (r + 1))
                sp.dma_start(
                    out[j * C : (j + 1) * C, :].rearrange("(p g) d -> p (g d)", p=128),
                    bufs[b][:, :],
                ).then_inc(w_sems[b], 16)
            for b in range(min(NB, JJ)):
                sp.wait_ge(w_sems[b], 16 * ((JJ - 1 - b) // NB + 1))

        @block.tensor
        def _(pe: bass.BassEngine):
            pe.wait_ge(s_in, 96)
            # TRN2 fatal hazard: PE writing a PSUM bank while DVE reads the
            # same bank crashes the core. Interleave per chunk and gate each
            # chunk's matmuls on DVE being completely done with the previous
            # chunk (s_e) so PE-writes and DVE-reads never overlap in PSUM.
            for c in range(TC):
                if c >= 1:
                    pe.wait_ge(s_e, c)
                # cumsum: chunk c rows = ones @ chunks<c + triu @ chunk c,
                # accumulated in increasing row order (exact sequential
                # prefix sum)
                for cp in range(c + 1):
                    mm = pe.matmul(
                        ps_cs[:, c, :],
                        (triu_sb if cp == c else ones_sb)[:, :],
                        cs_sb[:, cp, :],
                        start=(cp == 0),
                        stop=(cp == c),
                    )
                    if cp == c:
                        mm.then_inc(s_pe, 1)
                # interpolation + beta: [alpha, 1-alpha, 1] @ [I0; I1; beta]
                pe.matmul(
                    ps_in[:, c, :],
                    a3t_sb[:, c * 128 : (c + 1) * 128],
                    i3_sb[:, :],
                    start=True,
                    stop=True,
                ).then_inc(s_pi, 1)

        @block.scalar
        def _(act: bass.BassEngine):
            act.wait_ge(s_eps, 1)
            for c in range(TC):
                act.wait_ge(s_mv, c + 1)
                act.activation(
                    out=rstd_sb[:, c : c + 1],
                    in_=mv_sb[:, c, 1:2],
                    func=AF.Sqrt,
                    bias=eps_sb[:, 0:1],
                    scale=1.0,
                ).then_inc(s_rs, 1)

        @block.vector
        def _(dve: bass.BassEngine):
            # raw-bass DVE writes are posted: serialize each dependent step
            # with a sem (prologue-only, 8 iterations, cost is negligible)
            dv_cnt = [0]

            def step(inst):
                dv_cnt[0] += 1
                inst.then_inc(s_dv, 1)
                dve.wait_ge(s_dv, dv_cnt[0])

            dve.memset(eps_sb[:, :], EPS).then_inc(s_eps, 1)
            for c in range(TC):
                dve.wait_ge(s_pe, c + 1)
                step(dve.bn_stats(out=stats_sb[:, c, :], in_=ps_cs[:, c, :]))
                dve.bn_aggr(out=mv_sb[:, c, :], in_=stats_sb[:, c, :]).then_inc(s_mv, 1)
                dve.wait_ge(s_rs, c + 1)
                step(dve.reciprocal(out=rstd_sb[:, c : c + 1], in_=rstd_sb[:, c : c + 1]))
                step(
                    dve.tensor_scalar(
                        out=e_sb[:, c, :],
                        in0=ps_cs[:, c, :],
                        scalar1=mv_sb[:, c, 0:1],
                        scalar2=rstd_sb[:, c : c + 1],
                        op0=ALU.subtract,
                        op1=ALU.mult,
                    )
                )
                step(
                    dve.tensor_tensor(
                        out=e_sb[:, c, :],
                        in0=e_sb[:, c, :],
                        in1=gamma_sb[:, :],
                        op=ALU.mult,
                    )
                )
                dve.wait_ge(s_pi, c + 1)
                dve.tensor_tensor(
                    out=e_sb[:, c, :], in0=e_sb[:, c, :], in1=ps_in[:, c, :], op=ALU.add
                ).then_inc(s_e, 1)

        @block.gpsimd
        def _(gp: bass.BassGpSimd):
            gp.load_library(mlp)
            gp.wait_ge(s_ix, 16)
            gp.wait_ge(s_et, 16)
            for jj in range(JJ):
                j, b, r = jj % J, jj % NB, jj // NB
                q = j % NQ
                if r >= 1:
                    gp.wait_ge(w_sems[b], 16 * r)
                gp.dma_gather(
                    bufs[b][:, :].rearrange("p (g d) -> p g d", d=D),
                    etab[:, :],
                    idx_sb[:, (j // NQ) * (C // 16) : (j // NQ + 1) * (C // 16)],
                    C,
                    C,
                    D,
                    # single_packet concatenates all of an engine's descriptors
                    # into one packet; at C/16=400 descriptors per engine that
                    # blows the <=64-descriptor packet ceiling and kills the
                    # core. One packet per 512 B descriptor is required here.
                    single_packet=False,
                    queue_num=q,
                ).then_inc(g_sems[b], 16)

    nc.compile()
    return nc


_NC_CACHE: dict = {}


def _get_nc(rows: int, reps: int = 1) -> bass.Bass:
    if (rows, reps) not in _NC_CACHE:
        _NC_CACHE[(rows, reps)] = build_nc(rows, reps)
    return _NC_CACHE[(rows, reps)]


def prep_idx(flat: np.ndarray, rows: int) -> np.ndarray:
    """Permute + wrap one core's index stream for dma_gather.

    Gather flat position i = g*128 + p of chunk j must hold the index for
    output row jC + p*G + g, wrapped into the ucode's [16, n/16] layout
    (position i -> partition i%16, column i//16). Chunk j is processed by
    gpsimd core pair q = j % NQ, which reads indices from partitions
    [32q, 32q+32) only; chunk j's columns are packed at column offset
    (j // NQ) * C/16 of partition block q (two 16-partition replicas).
    """
    J = rows // C
    a = flat.reshape(J, 128, G).transpose(0, 2, 1).reshape(J, C)
    w = a.reshape(J, C // 16, 16).transpose(0, 2, 1)  # [J, 16, C/16]
    out = np.zeros((128, rows // 16 // NQ), np.int16)
    for j in range(J):
        q, col = j % NQ, (j // NQ) * (C // 16)
        blk = w[j].astype(np.int16)  # [16, C/16]
        out[32 * q : 32 * q + 16, col : col + C // 16] = blk
        out[32 * q + 16 : 32 * q + 32, col : col + C // 16] = blk
    return np.ascontiguousarray(out)


def host_consts(interpolation_embedding, ln_beta):
    triu = np.ascontiguousarray(np.triu(np.ones((128, 128), np.float32)))
    onesm = np.ones((128, 128), np.float32)
    n = np.arange(N, dtype=np.float32)
    alpha = (np.float32(N - 1) - n) / np.float32(N - 1)
    a3t = np.ascontiguousarray(
        np.stack([alpha, np.float32(1.0) - alpha, np.ones(N, np.float32)])
    )
    ie = np.asarray(interpolation_embedding, dtype=np.float32)
    beta = np.asarray(ln_beta, dtype=np.float32)
    i3 = np.ascontiguousarray(np.stack([ie[0], ie[1], beta]))
    return triu, onesm, a3t, i3


def _run(
    cumsum_embedding,
    interpolation_embedding,
    ln_gamma,
    ln_beta,
    index_tensor,
    reps: int = 1,
    **spmd_kwargs,
):
    nc = _get_nc(R, reps)
    triu, onesm, a3t, i3 = host_consts(interpolation_embedding, ln_beta)
    gammab = np.ascontiguousarray(
        np.tile(np.asarray(ln_gamma, np.float32)[None, :], (128, 1))
    )
    csemb = np.ascontiguousarray(np.asarray(cumsum_embedding, np.float32))
    flat = np.asarray(index_tensor).reshape(-1)

    in_maps = []
    for c in range(N_CORES):
        in_maps.append(
            {
                "csemb": csemb,
                "triu": triu,
                "onesm": onesm,
                "a3t": a3t,
                "i3": i3,
                "gammab": gammab,
                "idx16": prep_idx(flat[c * R : (c + 1) * R], R),
            }
        )

    res = run_bass_kernel_spmd(nc, in_maps, list(range(N_CORES)), **spmd_kwargs)
    outs = [
        np.asarray(res.results[c]["out"]).reshape(B // N_CORES, L, D)
        for c in range(N_CORES)
    ]
    return np.concatenate(outs, axis=0), res


def kernel(
    cumsum_embedding,
    interpolation_embedding,
    ln_gamma,
    ln_beta,
    index_tensor,
) -> np.ndarray:
    out, _ = _run(
        cumsum_embedding, interpolation_embedding, ln_gamma, ln_beta, index_tensor
    )
    return out
